# revision 69
# baseline (speedup 1.0000x reference)
"""Trainium2 Bass kernel for the mca_g2l sparse-attention module.

Sharding: head-parallel over 8 cores (1 head each). All on-device tensors are
feature-major ("^T": [feature, tokens]); attention is computed key-major
(S^T [keys, queries]) so the softmax denominators come from ones-matmuls and
the AV / ave-branch matmuls need no attention transpose.

Cross-core data movement (TWO collective hops, SPMD-symmetric, fp16 — relay
round trips dominate over payload bytes here, so fewer/fatter hops win):
  AG-x : AllGather of the fp16 x^T C-row shards (rebuilds full x^T everywhere)
  AR   : one AllReduce (add) of [attn_avg^T | raw_cls^T | raw_reg^T |
         row-split output-linear partials] — 5*N2 rows. After it, every core
         holds the full head-sums AND the complete summed output linears:
         masks, masked exps, renormalizers, and the whole ave branch are then
         computed locally (redundantly per core, ~free next to a hop), and
         the linears just need a bias add.
Raw v-v similarities are computed per-head locally in phase B (each core does
its own head for all 2048 keys) and head-summed by the AR, so no exchange of
normalized v is needed. Output linears are row-sharded (each core contracts
only its own x/x_ori feature rows — the AV outputs are never gathered); the
identical full linear result is written by every core and the host takes one
copy. Ave-branch output columns are head-sharded so `support` is the core's
own token-major v; the host assembles [512, 3072] from those slices.

Inputs ship as a single fp16 blob (~4.9 MB/core — per-exec input staging through
the axon relay costs ~0.67 ms/MB above ~5 MB, which dominated the f32 version;
below that knee staging hides under the ~4 ms relay dispatch floor). Matmuls run
fp16 x fp16 with f32 PSUM; softmax exps use a softmax-invariant logit shift of
-4 so unnormalized exp(logit) stays in fp16 range (|logit| <= ~12 here).
"""

import numpy as np

import concourse.bacc as bacc
import concourse.mybir as mybir
import concourse.tile as tile
from concourse.masks import make_identity

F32 = mybir.dt.float32
F32R = mybir.dt.float32r
BF16 = mybir.dt.bfloat16
F16 = mybir.dt.float16
AF = mybir.ActivationFunctionType

N_CORES = 8
N1 = 512
N2 = 2048
C = 1024
HD = 128
SCALE = 25.0
KT = N2 // 128          # 16 key tiles of 128
TT = N2 // 512          # 4 token tiles of 512
CC = C // 128           # 8 contraction chunks
MYK = N2 // N_CORES     # 256 keys owned per core after RS / A2A

EXP_SHIFT = -4.0                        # exp(logit-4): softmax-invariant shift
                                        # keeping unnormalized exps in fp16 range

# Single AllReduce payload (rows x N1, fp16): full head-sums of the attention
# average and raw sims plus the row-split output-linear partials. After the AR
# every core holds everything it needs: masks/exps/ave are computed locally,
# and the summed linears are complete on every core (host takes one copy).
AR_ATTN = 0
AR_RAWC = N2
AR_RAWR = 2 * N2
AR_LIN = 3 * N2                         # [cls 2048 | reg 2048] out-col rows
AR_ROWS = 5 * N2

# packed input blob layout (rows x 512 fp16). x^T is sharded: each core ships
# its 128 C-rows of xt_cls+xt_reg; an on-device AllGather rebuilds the full x^T.
XC0 = 0                                 # [1024, 512] = [256, 2048] x^T shard
WA0 = 1024                              # [C, 512]: qc | kc | vc | qr slices
WB0 = 2048                              # [512, 512]: kr (2 halves) | vr (2 halves)
MS0 = 2560                              # rows 0:4 score (4x512); rows 8:136
                                        # FULL biases [128, 32]: col ib*16+m
WL0 = 2696                              # [2C, 512]: wlin_cls | wlin_reg
BLOB_ROWS = 4744

RG = [list(range(N_CORES))]
B = ("cls", "reg")


def round_f32r(a: np.ndarray) -> np.ndarray:
    """Round-to-nearest-even at 11 explicit mantissa bits (= hardware f32r)."""
    u = np.ascontiguousarray(a, dtype=np.float32).view(np.uint32).astype(np.uint64)
    shift = np.uint64(12)
    bias = np.uint64((1 << 11) - 1)
    lsb = (u >> shift) & np.uint64(1)
    r = ((u + bias + lsb) >> shift) << shift
    return r.astype(np.uint32).view(np.float32).reshape(a.shape)


def build_nc(no_coll=False, phases=5):
    """Build the SPMD program (identical on every core; per-core data differs)."""
    nc = bacc.Bacc("TRN2", target_bir_lowering=False, debug=False,
                   num_devices=N_CORES)

    # ---- kernel I/O: single packed input blob + single packed output ----
    blob = nc.dram_tensor("blob", [BLOB_ROWS, 512], F16, kind="ExternalInput")
    out_t = nc.dram_tensor("out", [256 + 4 * C, 512], F16,
                           kind="ExternalOutput")
    bap = blob.ap()
    a_out = {"cls": out_t.ap()[0:128, :], "reg": out_t.ap()[128:256, :]}
    o_out = {"cls": out_t.ap()[256:256 + 2 * C, :],
             "reg": out_t.ap()[256 + 2 * C:256 + 4 * C, :]}

    with tile.TileContext(nc) as tc:
        with tc.tile_pool(name="dram", bufs=1, space="DRAM") as dramp, \
             tc.tile_pool(name="const", bufs=1) as constp, \
             tc.tile_pool(name="persist", bufs=1) as persist:

            # ---- internal DRAM for collectives ----
            agx_in = dramp.tile([2 * 128, N2], F16, name="agx_in")
            agx_out = dramp.tile([2 * C, N2], F16, name="agx_out",
                                 addr_space="Shared")
            ar_in = dramp.tile([AR_ROWS, N1], F16, name="ar_in")
            ar_out = dramp.tile([AR_ROWS, N1], F16, name="ar_out",
                                addr_space="Shared")

            # gather the full x^T from per-core shards first
            nc.sync.dma_start(agx_in[:],
                              bap[XC0:XC0 + 1024, :]
                              .rearrange("(r f) n -> r (f n)", f=4))
            nc.gpsimd.collective_compute(
                "AllGather", mybir.AluOpType.bypass, replica_groups=RG,
                ins=[agx_in.opt()], outs=[agx_out.opt()])

            # ---- constants ----
            ones_f = constp.tile([128, 1], F32, name="ones_f")
            nc.vector.memset(ones_f[:], 1.0)
            ones = constp.tile([128, 1], F16, name="ones")
            nc.vector.tensor_copy(ones[:], ones_f[:])
            ones8 = constp.tile([8, 1], F16, name="ones8")
            nc.vector.tensor_copy(ones8[:], ones_f[0:8, :])
            eshift = constp.tile([128, 1], F32, name="eshift")
            nc.vector.memset(eshift[:], EXP_SHIFT)
            # (identity matrix no longer needed: transposes go via DMA xbar)
            score16 = constp.tile([1, N2], F16, name="score16")
            nc.sync.dma_start(score16[:].rearrange("o (f n) -> o f n", f=4),
                              bap[MS0:MS0 + 4, :])
            score_s = constp.tile([1, N2], F32, name="score_s")
            nc.vector.tensor_copy(score_s[:], score16[:])
            # (biases are added on the host in assemble(), in f32)

            # ---- persistent SBUF (live until the end) ----
            vT512 = {b: persist.tile([128, N1], F16, name=f"vT512_{b}",
                                     tag=f"vT512_{b}") for b in B}
            XS = {b: persist.tile([128, N1], F16, name=f"XS_{b}",
                                  tag=f"XS_{b}") for b in B}
            vTok = {b: persist.tile([128, KT, 128], F16, name=f"vTok_{b}",
                                    tag=f"vTok_{b}") for b in B}

            # =========== Phases A+B under the k/v/q pool ===========
            with tc.tile_pool(name="ppool", bufs=1) as ppool:
                kS = {b: ppool.tile([128, KT, 128], F16, name=f"kS_{b}",
                                    tag=f"kS_{b}") for b in B}
                vN = {b: ppool.tile([128, KT, 128], F16, name=f"vN_{b}",
                                    tag=f"vN_{b}") for b in B}
                qN = {b: ppool.tile([128, N1], F16, name=f"qN_{b}",
                                    tag=f"qN_{b}") for b in B}

                # ---------------- Phase A: projections ----------------
                with tc.tile_pool(name="projw", bufs=1) as projw, \
                     tc.tile_pool(name="projx", bufs=2) as projx, \
                     tc.tile_pool(name="projtmp", bufs=2) as projtmp, \
                     tc.tile_pool(name="psA", bufs=3, space="PSUM") as psA, \
                     tc.tile_pool(name="psN", bufs=2, space="PSUM") as psN:

                    WA_SLOT = {("q", "cls"): 0, ("k", "cls"): 1,
                               ("v", "cls"): 2, ("q", "reg"): 3}
                    WB_SLOT = {("k", "reg"): 0, ("v", "reg"): 1}
                    for b in B:
                        w_s = {}
                        for t in ("q", "k", "v"):
                            w_s[t] = projw.tile([128, CC, HD], F16,
                                                name=f"w{t}", tag=f"w{t}")
                            if (t, b) in WA_SLOT:
                                j = WA_SLOT[t, b]
                                nc.sync.dma_start(
                                    w_s[t][:],
                                    bap[WA0:WA0 + C, j * 128:(j + 1) * 128]
                                    .rearrange("(c p) m -> p c m", p=128))
                            else:
                                j = WB_SLOT[t, b]
                                for hh in range(2):
                                    nc.sync.dma_start(
                                        w_s[t][:, 4 * hh:4 * hh + 4, :],
                                        bap[WB0:WB0 + 512,
                                            (2 * j + hh) * 128:
                                            (2 * j + hh + 1) * 128]
                                        .rearrange("(c p) m -> p c m", p=128))

                        for tt in range(TT):
                            xt_t = projx.tile([128, CC, 512], F16, name="xt",
                                              tag="xt")
                            ib = 0 if b == "cls" else 1
                            nc.sync.dma_start(
                                xt_t[:],
                                agx_out[:].rearrange("(c two p) n -> two p c n",
                                                     two=2, p=128)[ib]
                                [:, :, tt * 512:(tt + 1) * 512])

                            def proj(tname, xt_t=xt_t, w_s=w_s):
                                ps = psA.tile([128, 512], F32, name="proj",
                                              tag="proj")
                                for c in range(CC):
                                    nc.tensor.matmul(ps[:], w_s[tname][:, c, :],
                                                     xt_t[:, c, :],
                                                     start=(c == 0),
                                                     stop=(c == CC - 1))
                                return ps

                            def inv_norm(ps):
                                # 1/||col|| from a [128, 512] psum tile
                                sq = projtmp.tile([128, 512], F16, name="sq",
                                                  tag="sq")
                                nc.scalar.activation(sq[:], ps[:], AF.Square)
                                nsq = psN.tile([1, 512], F32, name="nsq", tag="nsq")
                                nc.tensor.matmul(nsq[:], ones[:], sq[:],
                                                 start=True, stop=True)
                                st = projtmp.tile([1, 512], F32, name="st", tag="st")
                                nc.scalar.activation(st[:], nsq[:], AF.Sqrt)
                                rt = projtmp.tile([1, 512], F32, name="rt", tag="rt")
                                nc.vector.reciprocal(rt[:], st[:])
                                return rt

                            def bcast(row):
                                bt = projtmp.tile([128, 512], F32, name="bc",
                                                  tag="bc")
                                nc.gpsimd.partition_broadcast(bt[:], row[:])
                                return bt

                            tsl = slice(tt * 4, (tt + 1) * 4)

                            # --- k: fold SCALE (and cls_score) and 1/|k| in ---
                            pk = proj("k")
                            rk = inv_norm(pk)
                            fk = projtmp.tile([1, 512], F32, name="fk", tag="fk")
                            nc.vector.tensor_scalar_mul(fk[:], rk[:], SCALE)
                            if b == "cls":
                                nc.vector.tensor_mul(
                                    fk[:], fk[:], score_s[:, tt * 512:(tt + 1) * 512])
                            nc.vector.tensor_mul(kS[b][:, tsl, :], pk[:], bcast(fk)[:])

                            # --- v: normalized copy + raw copy + transposes ---
                            pv = proj("v")
                            rv = inv_norm(pv)
                            nc.vector.tensor_mul(vN[b][:, tsl, :], pv[:], bcast(rv)[:])
                            vraw = (vT512[b] if tt == 0 else
                                    projtmp.tile([128, 512], F16, name="vraw",
                                                 tag="vraw"))
                            nc.scalar.activation(vraw[:], pv[:], AF.Copy)
                            for j in range(4):
                                nc.sync.dma_start_transpose(
                                    vTok[b][:, tt * 4 + j, :],
                                    vraw[:, j * 128:(j + 1) * 128])

                            # --- q (first token tile only) ---
                            if tt == 0:
                                pq = proj("q")
                                rq = inv_norm(pq)
                                nc.vector.tensor_mul(qN[b][:], pq[:], bcast(rq)[:])

                # ---------------- Phase B: attention + raw sims ----------------
                with tc.tile_pool(name="Ppool", bufs=1) as Ppool, \
                     tc.tile_pool(name="attnps", bufs=2, space="PSUM") as attnps, \
                     tc.tile_pool(name="accps", bufs=1, space="PSUM") as accps, \
                     tc.tile_pool(name="attntmp", bufs=2) as attntmp, \
                     tc.tile_pool(name="rhpool", bufs=1) as rhpool, \
                     tc.tile_pool(name="avgpool", bufs=3) as avgpool:
                    P = {b: Ppool.tile([128, KT, N1], F16, name=f"P_{b}",
                                       tag=f"P_{b}") for b in B}
                    xacc = {b: accps.tile([128, N1], F32, name=f"x_{b}",
                                          tag=f"x_{b}") for b in B}
                    dacc = {b: accps.tile([1, N1], F32, name=f"d_{b}",
                                          tag=f"d_{b}") for b in B}
                    for b in B:
                        for kt in range(KT):
                            s = attnps.tile([128, N1], F32, name="s", tag="s")
                            nc.tensor.matmul(s[:], kS[b][:, kt, :], qN[b][:],
                                             start=True, stop=True)
                            p_t = P[b][:, kt, :]
                            nc.scalar.activation(p_t, s[:], AF.Exp,
                                                 bias=eshift[:])
                            nc.tensor.matmul(dacc[b][:], ones[:], p_t,
                                             start=(kt == 0), stop=(kt == KT - 1))
                            # this head's raw v-v similarity for these keys;
                            # the RS head-sums it for the mask thresholds
                            rp = attnps.tile([128, N1], F32, name="rp", tag="rp")
                            nc.tensor.matmul(rp[:], vN[b][:, kt, :],
                                             vN[b][:, 0:4, :],
                                             start=True, stop=True)
                            rw = avgpool.tile([128, N1], F16, name="rw", tag="rw")
                            nc.scalar.activation(rw[:], rp[:], AF.Copy)
                            off = AR_RAWC if b == "cls" else AR_RAWR
                            nc.sync.dma_start(
                                ar_in[off + kt * 128:off + (kt + 1) * 128, :],
                                rw[:])

                    Rhalf = {}
                    for b in B:
                        d2 = attntmp.tile([1, N1], F32, name="d2", tag="d2")
                        nc.vector.tensor_scalar_mul(d2[:], dacc[b][:], 2.0)
                        rh = attntmp.tile([1, N1], F32, name="rh", tag="rh")
                        nc.vector.reciprocal(rh[:], d2[:])
                        Rhalf[b] = rhpool.tile([128, N1], F32, name=f"Rh_{b}",
                                               tag=f"Rh_{b}")
                        nc.gpsimd.partition_broadcast(Rhalf[b][:], rh[:])

                    # attn_avg^T = P_cls/(2 D_cls) + P_reg/(2 D_reg), bf16, to DRAM;
                    # x^T[b] = sum_kt vTok_b[kt] @ (P_cls'[kt] + P_reg'[kt])
                    for kt in range(KT):
                        for b in B:
                            nc.vector.tensor_mul(P[b][:, kt, :], P[b][:, kt, :],
                                                 Rhalf[b][:])
                        av = avgpool.tile([128, N1], F16, name="avg", tag="avg")
                        nc.vector.tensor_add(av[:], P["cls"][:, kt, :],
                                             P["reg"][:, kt, :])
                        nc.sync.dma_start(ar_in[kt * 128:(kt + 1) * 128, :],
                                          av[:])
                        for b in B:
                            for i2, b2 in enumerate(B):
                                nc.tensor.matmul(
                                    xacc[b][:], vTok[b][:, kt, :], P[b2][:, kt, :],
                                    start=(kt == 0 and i2 == 0),
                                    stop=(kt == KT - 1 and i2 == 1))
                    for b in B:
                        nc.scalar.activation(XS[b][:], xacc[b][:], AF.Copy)

            # ===== E1 partials (local): row-split W_lin on own x/x_ori =====
            # out[q, g] = sum_r [x | x_ori][q, r] W[r, g]; this core owns rows
            # r in {h*128..} (x half, = XS) and {C+h*128..} (x_ori half, =
            # vT512); partials reduce-scatter to the 256-col owner per branch.
            with tc.tile_pool(name="linp", bufs=1) as linp, \
                 tc.tile_pool(name="linps", bufs=4, space="PSUM") as linps, \
                 tc.tile_pool(name="lintmp", bufs=3) as lintmp:
                for ib, b in enumerate(B):
                    wl = linp.tile([128, 8, 512], F16, name=f"wl_{b}",
                                   tag=f"wl_{b}")
                    nc.sync.dma_start(
                        wl[:], bap[WL0 + ib * 1024:WL0 + (ib + 1) * 1024, :]
                        .rearrange("(x p) n -> p x n", p=128))
                    for m in range(16):
                        op_ = linps.tile([128, N1], F32, name="olin", tag="olin")
                        for k, rhs in ((0, XS[b]), (1, vT512[b])):
                            lhsT = wl[:, k * 4 + m // 4,
                                      (m % 4) * 128:(m % 4) * 128 + 128]
                            nc.tensor.matmul(op_[:], lhsT, rhs[:],
                                             start=(k == 0), stop=(k == 1))
                        o16 = lintmp.tile([128, N1], F16, name="o16", tag="o16")
                        nc.scalar.activation(o16[:], op_[:], AF.Copy)
                        row = AR_LIN + ib * 2 * C + m * 128
                        nc.sync.dma_start(ar_in[row:row + 128, :], o16[:])
            nc.gpsimd.collective_compute(
                "AllReduce", mybir.AluOpType.add, replica_groups=RG,
                ins=[ar_in.opt()], outs=[ar_out.opt()])

            # ===== Phase D (local): masks, masked exps, ave branch, linears =====
            with tc.tile_pool(name="vng", bufs=1) as vng, \
                 tc.tile_pool(name="dps", bufs=1, space="PSUM") as dps, \
                 tc.tile_pool(name="avetmp", bufs=2) as avetmp:
                asum = vng.tile([128, KT, N1], F16, name="asum")
                nc.sync.dma_start(
                    asum[:], ar_out[AR_ATTN:AR_ATTN + N2, :]
                    .rearrange("(t p) q -> p t q", p=128))
                raw = {}
                for off, b in ((AR_RAWC, "cls"), (AR_RAWR, "reg")):
                    raw[b] = vng.tile([128, KT, N1], F16, name=f"raw_{b}",
                                      tag=f"raw_{b}")
                    nc.sync.dma_start(
                        raw[b][:], ar_out[off:off + N2, :]
                        .rearrange("(t p) q -> p t q", p=128))
                msk = {b: vng.tile([128, KT, N1], F16, name=f"msk_{b}",
                                   tag=f"msk_{b}") for b in B}
                for b, thr in (("cls", 0.75), ("reg", 0.99)):
                    for t in range(KT):
                        nc.vector.tensor_scalar(
                            msk[b][:, t, :], raw[b][:, t, :], 1.0 / N_CORES, thr,
                            mybir.AluOpType.mult, mybir.AluOpType.is_gt)

                # ME[cls] = sim_mask * exp(attn_sum/H); ME[reg] = obj_mask * that
                ME = {"cls": vng.tile([128, KT, N1], F16, name="mes16"),
                      "reg": vng.tile([128, KT, N1], F16, name="meo16")}
                dp = {b: dps.tile([1, N1], F32, name=f"dp_{b}", tag=f"dp_{b}")
                      for b in B}
                for t in range(KT):
                    e_t = avetmp.tile([128, N1], F16, name="e_t", tag="e_t")
                    nc.scalar.activation(e_t[:], asum[:, t, :], AF.Exp,
                                         scale=1.0 / N_CORES)
                    nc.vector.tensor_mul(ME["cls"][:, t, :], e_t[:],
                                         msk["cls"][:, t, :])
                    nc.vector.tensor_mul(ME["reg"][:, t, :], ME["cls"][:, t, :],
                                         msk["reg"][:, t, :])
                    for b in B:
                        nc.tensor.matmul(dp[b][:], ones[:], ME[b][:, t, :],
                                         start=(t == 0), stop=(t == KT - 1))
                Rd = {}
                for b in B:
                    rr = avetmp.tile([1, N1], F32, name="rr", tag="rr")
                    nc.vector.reciprocal(rr[:], dp[b][:])
                    Rd[b] = avetmp.tile([128, N1], F32, name=f"Rd_{b}",
                                        tag=f"Rd_{b}")
                    nc.gpsimd.partition_broadcast(Rd[b][:], rr[:])
                for b in B:
                    # columns of this head; support = own token-major v
                    ap_ = dps.tile([128, N1], F32, name="avep", tag=f"avep_{b}")
                    for kt in range(KT):
                        nc.tensor.matmul(ap_[:], vTok[b][:, kt, :],
                                         ME[b][:, kt, :],
                                         start=(kt == 0), stop=(kt == KT - 1))
                    asb = avetmp.tile([128, N1], F16, name="asb", tag="asb")
                    nc.vector.tensor_mul(asb[:], ap_[:], Rd[b][:])
                    nc.sync.dma_start(a_out[b], asb[:])

            # ===== E1 finalize: ship the all-reduced linear sums verbatim =====
            # (identical on every core; host takes one copy and adds the bias)
            nc.sync.dma_start(out_t.ap()[256:256 + 4 * C, :],
                              ar_out[AR_LIN:AR_LIN + 4 * C, :])

    nc.finalize()
    return nc


def make_in_maps(inputs: dict) -> list[dict]:
    """Host-side staging: pack per-core slices into one pre-rounded blob."""
    x_cls = np.asarray(inputs["x_cls"], np.float32)[0]      # [N2, C]
    x_reg = np.asarray(inputs["x_reg"], np.float32)[0]
    cls_score = np.asarray(inputs["cls_score"], np.float32)
    W_q = {"cls": np.asarray(inputs["W_q_cls"], np.float32),
           "reg": np.asarray(inputs["W_q_reg"], np.float32)}
    W_kv = {"cls": np.asarray(inputs["W_kv_cls"], np.float32),
            "reg": np.asarray(inputs["W_kv_reg"], np.float32)}
    W_l = {"cls": np.asarray(inputs["W_lin"], np.float32),
           "reg": np.asarray(inputs["W_lin_reg"], np.float32)}
    b_l = {"cls": np.asarray(inputs["b_lin"], np.float32),
           "reg": np.asarray(inputs["b_lin_reg"], np.float32)}

    xt = {b: np.ascontiguousarray(x.T).astype(np.float16)
          for b, x in (("cls", x_cls), ("reg", x_reg))}

    in_maps = []
    for h in range(N_CORES):
        hs = slice(h * HD, (h + 1) * HD)
        vs = slice(C + h * HD, C + (h + 1) * HD)
        blob = np.zeros((BLOB_ROWS, 512), np.float16)
        shard = np.concatenate([xt["cls"][h * HD:(h + 1) * HD],
                                xt["reg"][h * HD:(h + 1) * HD]], 0)
        blob[XC0:XC0 + 1024] = shard.reshape(1024, 512)
        wa = np.concatenate([W_q["cls"][:, hs], W_kv["cls"][:, hs],
                             W_kv["cls"][:, vs], W_q["reg"][:, hs]], 1)
        blob[WA0:WA0 + C] = wa.astype(np.float16)
        kr, vr = W_kv["reg"][:, hs], W_kv["reg"][:, vs]
        wb = np.concatenate([kr[0:512], kr[512:1024],
                             vr[0:512], vr[512:1024]], 1)
        blob[WB0:WB0 + 512] = wb.astype(np.float16)
        blob[MS0:MS0 + 4] = cls_score.reshape(4, 512).astype(np.float16)
        for i, b in enumerate(B):
            blob[MS0 + 8:MS0 + 136, i * 16:(i + 1) * 16] = \
                b_l[b].reshape(16, 128).T.astype(np.float16)
        # W_lin row-slices for the reduce-scattered output linears: this core
        # contracts rows {h*128..} (x half) and {C+h*128..} (x_ori half).
        wl_rows = np.r_[h * 128:(h + 1) * 128, C + h * 128:C + (h + 1) * 128]
        for ib, b in enumerate(B):
            ws = W_l[b][wl_rows, :]                       # [256, 2C]
            arr = ws.reshape(2, 128, 4, 4, 128).transpose(0, 2, 1, 3, 4)
            blob[WL0 + ib * 1024:WL0 + (ib + 1) * 1024] = \
                arr.reshape(1024, 512).astype(np.float16)
        in_maps.append({"blob": blob})
    return in_maps


def assemble(results: list[dict],
             biases: dict[str, np.ndarray]) -> tuple[np.ndarray, np.ndarray]:
    """Host-side gather: per-core ave slices + one copy of the linears + bias."""
    feats = []
    for i, b in enumerate(B):
        ave = np.concatenate(
            [results[c]["out"][i * 128:(i + 1) * 128].T
             for c in range(N_CORES)], 1)
        out = (results[0]["out"][256 + i * 2 * C:256 + (i + 1) * 2 * C].T
               .astype(np.float32) + biases[b][None, :])
        feats.append(np.concatenate([ave.astype(np.float32), out], 1))
    return feats[0], feats[1]


_CACHE = {}


def get_nc():
    if "nc" not in _CACHE:
        _CACHE["nc"] = build_nc()
    return _CACHE["nc"]


class _Runner:
    """Cached jitted SPMD executor (mirrors bass2jax.run_bass_via_pjrt)."""

    def __init__(self, nc):
        import jax
        from jax.sharding import Mesh, PartitionSpec
        from jax.experimental.shard_map import shard_map
        from concourse.bass2jax import (_bass_exec_p, install_neuronx_cc_hook,
                                        partition_id_tensor)
        install_neuronx_cc_hook()
        self.jax = jax
        pname = nc.partition_id_tensor.name if nc.partition_id_tensor else None
        in_names, out_names, out_avals, zero_outs = [], [], [], []
        for alloc in nc.m.functions[0].allocations:
            if not isinstance(alloc, mybir.MemoryLocationSet):
                continue
            name = alloc.memorylocations[0].name
            if alloc.kind == "ExternalInput":
                if name != pname:
                    in_names.append(name)
            elif alloc.kind == "ExternalOutput":
                out_names.append(name)
                shape = tuple(alloc.tensor_shape)
                dtype = mybir.dt.np(alloc.dtype)
                out_avals.append(jax.core.ShapedArray(shape, dtype))
                zero_outs.append(np.zeros(shape, dtype))
        self.in_names, self.out_names = in_names, out_names
        self.out_avals, self.zero_outs = out_avals, zero_outs
        n_params, n_outs = len(in_names), len(out_names)
        all_in = in_names + out_names + ([pname] if pname else [])

        def _body(*args):
            operands = list(args)
            if pname is not None:
                operands.append(partition_id_tensor())
            return tuple(_bass_exec_p.bind(
                *operands, out_avals=tuple(out_avals), in_names=tuple(all_in),
                out_names=tuple(out_names), lowering_input_output_aliases=(),
                sim_require_finite=True, sim_require_nnan=True, nc=nc))

        devices = jax.devices()[:N_CORES]
        mesh = Mesh(np.asarray(devices), ("core",))
        self.fn = jax.jit(
            shard_map(_body, mesh=mesh,
                      in_specs=(PartitionSpec("core"),) * (n_params + n_outs),
                      out_specs=(PartitionSpec("core"),) * n_outs,
                      check_rep=False),
            keep_unused=True)

    def __call__(self, in_maps):
        n = N_CORES
        concat_in = [np.concatenate([np.asarray(in_maps[c][k]) for c in range(n)], 0)
                     for k in self.in_names]
        concat_zeros = [np.zeros((n * z.shape[0], *z.shape[1:]), z.dtype)
                        for z in self.zero_outs]
        outs = self.fn(*concat_in, *concat_zeros)
        self.jax.block_until_ready(outs)
        return [{name: np.asarray(outs[i]).reshape(n, *self.out_avals[i].shape)[c]
                 for i, name in enumerate(self.out_names)}
                for c in range(n)]


def get_runner():
    if "runner" not in _CACHE:
        _CACHE["runner"] = _Runner(get_nc())
    return _CACHE["runner"]


def kernel(**inputs) -> tuple[np.ndarray, np.ndarray]:
    results = get_runner()(make_in_maps(inputs))
    biases = {"cls": np.asarray(inputs["b_lin"], np.float32),
              "reg": np.asarray(inputs["b_lin_reg"], np.float32)}
    return assemble(results, biases)

